# revision 9
# baseline (speedup 1.0000x reference)
"""TP-8 Trainium2 Bass kernel for a CogVLM dual-expert decoder layer.

Layer (reference semantics):
  x   = rmsnorm(hidden) ; qkv per-expert (vision = tokens 0:1024, lang = 1024:2048)
  attn = causal MHA (32 heads, hd 128) with llama rotary
  res  = hidden + per-expert dense(attn)
  out  = per-expert SwiGLU MLP(rmsnorm(res))
  returns (out, res)

Sharding (tensor-parallel over 8 cores):
  QKV / gate_up column-sharded, attn-dense / down row-sharded per expert;
  activations AllGather'd in transposed [hid, tok] layout, partial sums
  ReduceScatter'd per 512-wide hidden slice; norms computed on each core's
  256-token chunk. All matmuls bf16 with fp32 PSUM accumulation.
"""
import sys, os

for _p in ("/opt/trn_rl_repo", "/root/.axon_site/_ro/trn_rl_repo"):
    if os.path.isdir(_p) and _p not in sys.path:
        sys.path.insert(0, _p)

import numpy as np
import ml_dtypes

import concourse.bass as bass
import concourse.bacc as bacc
import concourse.tile as tile
import concourse.mybir as mybir
from concourse.bass_utils import run_bass_kernel_spmd

bf16 = ml_dtypes.bfloat16
FP32 = mybir.dt.float32
BF16 = mybir.dt.bfloat16
AL = mybir.AluOpType
AF = mybir.ActivationFunctionType

NCORES = 8
S, H, I = 2048, 4096, 11008
SC = S // NCORES          # 256 tokens per core
HD = 128
QC = 512                  # q (and k, v) columns per core = 4 heads * 128
EPS = 1e-5
SCALE = 1.0 / np.sqrt(HD)
NIT = 11                  # padded icol tiles per core (11008/128 = 86 -> 6x11 + 2x10)
RG = [list(range(NCORES))]


def _icol_range(c):
    t0 = c * 11 if c < 6 else 66 + (c - 6) * 10
    nt = 11 if c < 6 else 10
    return t0 * 128, nt * 128


def build():
    nc = bacc.Bacc("TRN2", target_bir_lowering=False, debug=False,
                   num_devices=NCORES, enable_asserts=False)

    hid_d = nc.dram_tensor("hid", [SC, H], FP32, kind="ExternalInput")
    wqk_d = nc.dram_tensor("wqk", [2, H, 2 * QC], BF16, kind="ExternalInput")
    wv_d = nc.dram_tensor("wv", [2, H, QC], BF16, kind="ExternalInput")
    wo_d = nc.dram_tensor("wo", [2, QC, H], BF16, kind="ExternalInput")
    wgu_d = nc.dram_tensor("wgu", [2, H, 2 * NIT * 128], BF16, kind="ExternalInput")
    wd_d = nc.dram_tensor("wd", [2, NIT * 128, H], BF16, kind="ExternalInput")
    cos_d = nc.dram_tensor("cosT", [HD, S], BF16, kind="ExternalInput")
    sin_d = nc.dram_tensor("sinTs", [HD, S], BF16, kind="ExternalInput")
    msk_d = nc.dram_tensor("masks", [4, 128, 512], BF16, kind="ExternalInput")
    idn_d = nc.dram_tensor("ident", [128, 128], BF16, kind="ExternalInput")

    res_d = nc.dram_tensor("res_chunk", [SC, H], FP32, kind="ExternalOutput")
    outc_d = nc.dram_tensor("out_chunks", [8, SC, 512], FP32, kind="ExternalOutput")

    agin1 = nc.dram_tensor("agin1", [H, SC], BF16)
    agin2 = nc.dram_tensor("agin2", [H, SC], BF16)
    xnT_full = nc.dram_tensor("xnT_full", [NCORES, H, SC], BF16, addr_space="Shared")
    x2T_full = nc.dram_tensor("x2T_full", [NCORES, H, SC], BF16, addr_space="Shared")
    rsi_o = nc.dram_tensor("rsi_o", [8, S, 512], FP32)
    rso_o = nc.dram_tensor("rso_o", [8, SC, 512], FP32)
    rsi_d = nc.dram_tensor("rsi_d", [8, S, 512], FP32)
    rso_d = nc.dram_tensor("rso_d", [8, SC, 512], FP32)

    with tile.TileContext(nc) as tc:
        from contextlib import ExitStack
        ctx = ExitStack()
        with ctx:
            constp = ctx.enter_context(tc.tile_pool(name="constp", bufs=1))
            bigA = ctx.enter_context(tc.tile_pool(name="bigA", bufs=1))
            bigB = ctx.enter_context(tc.tile_pool(name="bigB", bufs=1))
            attp = ctx.enter_context(tc.tile_pool(name="attp", bufs=1))
            midp = ctx.enter_context(tc.tile_pool(name="midp", bufs=1))
            hidp = ctx.enter_context(tc.tile_pool(name="hidp", bufs=2))
            scr = ctx.enter_context(tc.tile_pool(name="scr", bufs=4))
            bfs = ctx.enter_context(tc.tile_pool(name="bfs", bufs=4))
            wst = ctx.enter_context(tc.tile_pool(name="wst", bufs=4))
            lst = ctx.enter_context(tc.tile_pool(name="lst", bufs=4))
            tiny = ctx.enter_context(tc.tile_pool(name="tiny", bufs=4))
            psp = ctx.enter_context(tc.tile_pool(name="psp", bufs=8, space="PSUM"))

            mask_sb = constp.tile([128, 4, 512], BF16, name="mask_sb")
            nc.sync.dma_start(mask_sb[:], msk_d[:].rearrange("m p q -> p m q"))
            idn_sb = constp.tile([128, 128], BF16, name="idn_sb")
            nc.sync.dma_start(idn_sb[:], idn_d[:])
            ones_sb = constp.tile([128, 1], BF16, name="ones_sb")
            nc.vector.memset(ones_sb[:], 1.0)
            onesM_sb = constp.tile([1, 128], BF16, name="onesM_sb")
            nc.vector.memset(onesM_sb[:], 1.0)
            cos_sb = constp.tile([128, S], BF16, name="cos_sb")
            nc.sync.dma_start(cos_sb[:], cos_d[:])
            sin_sb = constp.tile([128, S], BF16, name="sin_sb")
            nc.sync.dma_start(sin_sb[:], sin_d[:])

            # ---- stage A: ln1 on own chunk, transpose, AllGather ----
            hidA = []
            for tt in range(2):
                h_t = hidp.tile([128, H], FP32, tag="hid", name=f"hidA{tt}")
                nc.sync.dma_start(h_t[:], hid_d[128 * tt:128 * (tt + 1), :])
                hidA.append(h_t)
            xc1 = midp.tile([128, 32, SC], BF16, tag="mid", name="xc1")
            for tt in range(2):
                x = hidA[tt]
                sq = bigA.tile([128, H], BF16, tag="bigA", name=f"sqA{tt}")
                ssq = tiny.tile([128, 1], FP32, tag="ssq", name=f"ssqA{tt}")
                nc.scalar.activation(sq[:], x[:], AF.Square, accum_out=ssq[:])
                t1 = tiny.tile([128, 1], FP32, tag="ssq", name=f"lt1A{tt}")
                nc.vector.tensor_scalar(t1[:], ssq[:], 1.0 / H, EPS, AL.mult, AL.add)
                t2 = tiny.tile([128, 1], FP32, tag="ssq", name=f"lt2A{tt}")
                nc.scalar.sqrt(t2[:], t1[:])
                rstd = tiny.tile([128, 1], FP32, tag="ssq", name=f"rstdA{tt}")
                nc.vector.reciprocal(rstd[:], t2[:])
                xb = bigA.tile([128, H], BF16, tag="bigA", name=f"xbA{tt}")
                nc.scalar.activation(xb[:], x[:], AF.Copy, scale=rstd[:])
                for ht in range(32):
                    pt = psp.tile([128, 128], BF16, tag="ps",
                                  name=f"ptA{tt}_{ht}")
                    nc.tensor.transpose(pt[:], xb[:, 128 * ht:128 * (ht + 1)], idn_sb[:])
                    nc.vector.tensor_copy(xc1[:, ht, 128 * tt:128 * (tt + 1)], pt[:])
            nc.sync.dma_start(agin1[:].rearrange("(ht p) t -> p ht t", p=128), xc1[:])
            nc.gpsimd.collective_compute(
                "AllGather", AL.bypass, replica_groups=RG,
                ins=[agin1[:].opt()], outs=[xnT_full[:].opt()])

            # ---- stage B1: Q^T, K^T (weights stationary, xnT moving) ----
            qkT = bigB.tile([128, 8, S], BF16, tag="bigB", name="qkT")
            for e in range(2):
                wqk_sb = bigA.tile([128, 32, 2 * QC], BF16, tag="bigA",
                                   name=f"wqk{e}")
                nc.sync.dma_start(wqk_sb[:],
                                  wqk_d[e].rearrange("(ht p) n -> p ht n", p=128))
                for qc in range(2):
                    g = 2 * e + qc
                    ps = [psp.tile([128, 512], FP32, tag="ps", name=f"psB{g}_{j}")
                          for j in range(8)]
                    for ht in range(32):
                        xt = bfs.tile([128, 512], BF16, tag="bfs", name=f"xtB{g}_{ht}")
                        nc.sync.dma_start(
                            xt[:],
                            xnT_full[2 * g:2 * g + 2, 128 * ht:128 * (ht + 1), :]
                            .rearrange("c p t -> p c t"))
                        for j in range(8):
                            nc.tensor.matmul(ps[j][:],
                                             wqk_sb[:, ht, 128 * j:128 * (j + 1)],
                                             xt[:], start=(ht == 0), stop=(ht == 31))
                    tsl = slice(512 * g, 512 * (g + 1))
                    for j in range(8):
                        t1 = scr.tile([128, 512], FP32, tag="scr", name=f"t1B{g}_{j}")
                        nc.vector.tensor_tensor(t1[:], ps[j][:], cos_sb[:, tsl], op=AL.mult)
                        t2 = scr.tile([128, 512], FP32, tag="scr", name=f"t2B{g}_{j}")
                        nc.vector.tensor_tensor(t2[0:64, :], ps[j][64:128, :],
                                                sin_sb[0:64, tsl], op=AL.mult)
                        nc.vector.tensor_tensor(t2[64:128, :], ps[j][0:64, :],
                                                sin_sb[64:128, tsl], op=AL.mult)
                        nc.vector.tensor_tensor(qkT[:, j, tsl], t1[:], t2[:], op=AL.add)

            # ---- stage B2: V (xnT stationary, weights moving) ----
            v_sb = midp.tile([128, 16, 512], BF16, tag="mid", name="v_sb")
            for e in range(2):
                wv_sb = bigA.tile([128, 32, 512], BF16, tag="bigA", name=f"wv{e}")
                nc.sync.dma_start(wv_sb[:],
                                  wv_d[e].rearrange("(ht p) n -> p ht n", p=128))
                for tti in range(8):
                    tt = 8 * e + tti
                    pv = psp.tile([128, 512], FP32, tag="ps", name=f"psV{tt}")
                    for ht in range(32):
                        lt = lst.tile([128, 128], BF16, tag="lst", name=f"ltV{tt}_{ht}")
                        nc.sync.dma_start(
                            lt[:],
                            xnT_full[tt // 2, 128 * ht:128 * (ht + 1),
                                     128 * (tt % 2):128 * (tt % 2) + 128])
                        nc.tensor.matmul(pv[:], lt[:], wv_sb[:, ht, :],
                                         start=(ht == 0), stop=(ht == 31))
                    nc.vector.tensor_copy(v_sb[:, tt, :], pv[:])

            # ---- stage C: causal attention, 4 heads ----
            attnT = attp.tile([128, 4, S], BF16, tag="att", name="attnT")
            for h in range(4):
                for qch in range(4):
                    qsl = slice(512 * qch, 512 * (qch + 1))
                    pv = psp.tile([128, 512], FP32, tag="ps", name=f"psPV{h}_{qch}")
                    psm = psp.tile([1, 512], FP32, tag="ps", name=f"psSM{h}_{qch}")
                    nkt = 4 * (qch + 1)
                    for kt in range(nkt):
                        ss = psp.tile([128, 512], FP32, tag="ps",
                                      name=f"psS{h}_{qch}_{kt}")
                        nc.tensor.matmul(ss[:],
                                         qkT[:, 4 + h, 128 * kt:128 * (kt + 1)],
                                         qkT[:, h, qsl], start=True, stop=True)
                        pb = bfs.tile([128, 512], BF16, tag="bfs",
                                      name=f"pb{h}_{qch}_{kt}")
                        nc.scalar.activation(pb[:], ss[:], AF.Exp, scale=float(SCALE))
                        if kt >= 4 * qch:
                            nc.vector.tensor_tensor(pb[:], pb[:],
                                                    mask_sb[:, kt - 4 * qch, :],
                                                    op=AL.mult)
                        nc.tensor.matmul(pv[:], v_sb[:, kt, 128 * h:128 * (h + 1)],
                                         pb[:], start=(kt == 0), stop=(kt == nkt - 1))
                        nc.tensor.matmul(psm[:], ones_sb[:], pb[:],
                                         start=(kt == 0), stop=(kt == nkt - 1))
                    rec = tiny.tile([1, 512], BF16, tag="rec", name=f"rec{h}_{qch}")
                    with nc.allow_low_precision(reason="softmax denom bf16 ok"):
                        nc.vector.reciprocal(rec[:], psm[:])
                    pbc = psp.tile([128, 512], FP32, tag="ps", name=f"psBC{h}_{qch}")
                    nc.tensor.matmul(pbc[:], onesM_sb[:], rec[:], start=True, stop=True)
                    rbc = scr.tile([128, 512], FP32, tag="scr", name=f"rbc{h}_{qch}")
                    nc.vector.tensor_copy(rbc[:], pbc[:])
                    nc.vector.tensor_tensor(attnT[:, h, qsl], pv[:], rbc[:], op=AL.mult)

            # ---- stage D: attn dense partials + per-hid-slice ReduceScatter ----
            for hc in range(8):
                hsl = slice(512 * hc, 512 * (hc + 1))
                for e in range(2):
                    wot = [wst.tile([128, 512], BF16, tag="wst",
                                    name=f"wo{hc}_{e}_{ac}") for ac in range(4)]
                    for ac in range(4):
                        nc.sync.dma_start(wot[ac][:],
                                          wo_d[e, 128 * ac:128 * (ac + 1), hsl])
                    for tts in range(8):
                        tt = 8 * e + tts
                        po = psp.tile([128, 512], FP32, tag="ps",
                                      name=f"psO{hc}_{tt}")
                        for ac in range(4):
                            nc.tensor.matmul(po[:],
                                             attnT[:, ac, 128 * tt:128 * (tt + 1)],
                                             wot[ac][:], start=(ac == 0),
                                             stop=(ac == 3))
                        eo = scr.tile([128, 512], FP32, tag="scr",
                                      name=f"eoO{hc}_{tt}")
                        nc.vector.tensor_copy(eo[:], po[:])
                        nc.sync.dma_start(rsi_o[hc, 128 * tt:128 * (tt + 1), :], eo[:])
                nc.gpsimd.collective_compute(
                    "ReduceScatter", AL.add, replica_groups=RG,
                    ins=[rsi_o[hc].opt()], outs=[rso_o[hc].opt()])

            # ---- residual + ln2 + AllGather ----
            xc2 = midp.tile([128, 32, SC], BF16, tag="mid", name="xc2")
            for tt in range(2):
                rh = hidp.tile([128, H], FP32, tag="hid", name=f"hidD{tt}")
                nc.sync.dma_start(rh[:], hid_d[128 * tt:128 * (tt + 1), :])
                for hc in range(8):
                    rt = scr.tile([128, 512], FP32, tag="scr", name=f"rt{tt}_{hc}")
                    nc.sync.dma_start(rt[:], rso_o[hc, 128 * tt:128 * (tt + 1), :])
                    nc.vector.tensor_tensor(rh[:, 512 * hc:512 * (hc + 1)],
                                            rh[:, 512 * hc:512 * (hc + 1)],
                                            rt[:], op=AL.add)
                nc.sync.dma_start(res_d[128 * tt:128 * (tt + 1), :], rh[:])
                sq = bigA.tile([128, H], BF16, tag="bigA", name=f"sqD{tt}")
                ssq = tiny.tile([128, 1], FP32, tag="ssq", name=f"ssqD{tt}")
                nc.scalar.activation(sq[:], rh[:], AF.Square, accum_out=ssq[:])
                t1 = tiny.tile([128, 1], FP32, tag="ssq", name=f"lt1D{tt}")
                nc.vector.tensor_scalar(t1[:], ssq[:], 1.0 / H, EPS, AL.mult, AL.add)
                t2 = tiny.tile([128, 1], FP32, tag="ssq", name=f"lt2D{tt}")
                nc.scalar.sqrt(t2[:], t1[:])
                rstd = tiny.tile([128, 1], FP32, tag="ssq", name=f"rstdD{tt}")
                nc.vector.reciprocal(rstd[:], t2[:])
                xb = bigA.tile([128, H], BF16, tag="bigA", name=f"xbD{tt}")
                nc.scalar.activation(xb[:], rh[:], AF.Copy, scale=rstd[:])
                for ht in range(32):
                    pt = psp.tile([128, 128], BF16, tag="ps",
                                  name=f"ptD{tt}_{ht}")
                    nc.tensor.transpose(pt[:], xb[:, 128 * ht:128 * (ht + 1)], idn_sb[:])
                    nc.vector.tensor_copy(xc2[:, ht, 128 * tt:128 * (tt + 1)], pt[:])
            nc.sync.dma_start(agin2[:].rearrange("(ht p) t -> p ht t", p=128), xc2[:])
            nc.gpsimd.collective_compute(
                "AllGather", AL.bypass, replica_groups=RG,
                ins=[agin2[:].opt()], outs=[x2T_full[:].opt()])

            # ---- stage E: SwiGLU MLP ----
            h_sb = bigB.tile([128, 2, NIT, 1024], BF16, tag="bigB", name="h_sb")
            for e in range(2):
                x2 = bigA.tile([128, 32, 4, SC], BF16, tag="bigA", name=f"x2{e}")
                for cch in range(4):
                    nc.sync.dma_start(
                        x2[:, :, cch, :],
                        x2T_full[4 * e + cch]
                        .rearrange("(ht p) t -> p ht t", p=128))
                for ig in range(6):
                    ntl = 4 if ig < 5 else 2
                    pg = [psp.tile([128, 512], FP32, tag="ps",
                                   name=f"psG{e}_{ig}_{i}") for i in range(2 * ntl)]
                    for ht in range(32):
                        wt = wst.tile([128, 512], BF16, tag="wst",
                                      name=f"wgu{e}_{ig}_{ht}")
                        nc.sync.dma_start(
                            wt[:, 0:128 * ntl],
                            wgu_d[e, 128 * ht:128 * (ht + 1),
                                  512 * ig:512 * ig + 128 * ntl])
                        for t4 in range(ntl):
                            for half in range(2):
                                nc.tensor.matmul(
                                    pg[2 * t4 + half][:],
                                    wt[:, 128 * t4:128 * (t4 + 1)],
                                    x2[:, ht, 2 * half:2 * half + 2, :],
                                    start=(ht == 0), stop=(ht == 31))
                    for pr in range(ntl // 2):
                        ii = 2 * ig + pr
                        for half in range(2):
                            gps = pg[2 * (2 * pr) + half]
                            ups = pg[2 * (2 * pr + 1) + half]
                            gs = bfs.tile([128, 512], BF16, tag="bfs",
                                          name=f"gs{e}_{ii}_{half}")
                            nc.scalar.activation(gs[:], gps[:], AF.Silu)
                            nc.vector.scalar_tensor_tensor(
                                h_sb[:, e, ii, 512 * half:512 * (half + 1)],
                                ups[:], 1.0, gs[:], AL.mult, AL.mult)

            for hc in range(8):
                hsl = slice(512 * hc, 512 * (hc + 1))
                for e in range(2):
                    pd = [psp.tile([128, 512], FP32, tag="ps",
                                   name=f"psD{hc}_{e}_{t}") for t in range(8)]
                    for it in range(NIT):
                        wt = wst.tile([128, 512], BF16, tag="wst",
                                      name=f"wd{hc}_{e}_{it}")
                        nc.sync.dma_start(wt[:],
                                          wd_d[e, 128 * it:128 * (it + 1), hsl])
                        for tts in range(8):
                            nc.tensor.matmul(pd[tts][:],
                                             h_sb[:, e, it, 128 * tts:128 * (tts + 1)],
                                             wt[:], start=(it == 0),
                                             stop=(it == NIT - 1))
                    for tts in range(8):
                        ed = scr.tile([128, 512], FP32, tag="scr",
                                      name=f"ed{hc}_{e}_{tts}")
                        nc.vector.tensor_copy(ed[:], pd[tts][:])
                        nc.sync.dma_start(
                            rsi_d[hc, 128 * (8 * e + tts):128 * (8 * e + tts + 1), :],
                            ed[:])
                nc.gpsimd.collective_compute(
                    "ReduceScatter", AL.add, replica_groups=RG,
                    ins=[rsi_d[hc].opt()], outs=[rso_d[hc].opt()])
                nc.sync.dma_start(outc_d[hc], rso_d[hc])

    nc.compile()
    return nc


def prepare_in_maps(inputs):
    f32 = lambda x: np.asarray(x, dtype=np.float32)
    hidden = f32(inputs["hidden_states"]).reshape(S, H)
    cos, sin = f32(inputs["cos"]), f32(inputs["sin"])
    w_ln1, w_ln2 = f32(inputs["w_ln1"]), f32(inputs["w_ln2"])
    wqkv = [f32(inputs["wqkv_v"]), f32(inputs["wqkv_l"])]
    wo_ = [f32(inputs["wo_v"]), f32(inputs["wo_l"])]
    wgu_ = [f32(inputs["wgu_v"]), f32(inputs["wgu_l"])]
    wd_ = [f32(inputs["wd_v"]), f32(inputs["wd_l"])]

    cosT = np.ascontiguousarray(cos.T).astype(bf16)
    sinTs = sin.T.copy()
    sinTs[0:64] *= -1.0
    sinTs = np.ascontiguousarray(sinTs).astype(bf16)

    masks = np.zeros((4, 128, 512), np.float32)
    kk = np.arange(128)[:, None]
    qq = np.arange(512)[None, :]
    for ri in range(4):
        masks[ri] = (kk + 128 * ri <= qq)
    masks = masks.astype(bf16)
    ident = np.eye(128, dtype=np.float32).astype(bf16)

    wqkv_s = [w_ln1[:, None] * w for w in wqkv]
    in_maps = []
    for c in range(NCORES):
        qsl = slice(512 * c, 512 * (c + 1))
        ksl = slice(4096 + 512 * c, 4096 + 512 * (c + 1))
        vsl = slice(8192 + 512 * c, 8192 + 512 * (c + 1))
        wqk = np.stack([np.concatenate([wqkv_s[e][:, qsl], wqkv_s[e][:, ksl]],
                                       axis=1).astype(bf16) for e in range(2)])
        wv = np.stack([wqkv_s[e][:, vsl].astype(bf16) for e in range(2)])
        wo = np.stack([wo_[e][qsl, :].astype(bf16) for e in range(2)])
        i0, ni = _icol_range(c)
        wgu = np.zeros((2, H, 2 * NIT * 128), bf16)
        wd = np.zeros((2, NIT * 128, H), bf16)
        for e in range(2):
            g = (w_ln2[:, None] * wgu_[e])[:, i0:i0 + ni]
            u = (w_ln2[:, None] * wgu_[e])[:, 11008 + i0:11008 + i0 + ni]
            for i in range(ni // 128):
                wgu[e][:, 256 * i:256 * i + 128] = g[:, 128 * i:128 * (i + 1)].astype(bf16)
                wgu[e][:, 256 * i + 128:256 * (i + 1)] = u[:, 128 * i:128 * (i + 1)].astype(bf16)
            wd[e][:ni] = wd_[e][i0:i0 + ni, :].astype(bf16)
        in_maps.append({
            "hid": np.ascontiguousarray(hidden[SC * c:SC * (c + 1)]),
            "wqk": wqk, "wv": wv, "wo": wo, "wgu": wgu, "wd": wd,
            "cosT": cosT, "sinTs": sinTs, "masks": masks, "ident": ident,
        })
    return in_maps


_NC = None


def kernel(**inputs):
    global _NC
    if _NC is None:
        _NC = build()
    in_maps = prepare_in_maps(inputs)
    res = run_bass_kernel_spmd(_NC, in_maps, core_ids=list(range(NCORES)))
    out = np.zeros((S, H), np.float32)
    resid = np.zeros((S, H), np.float32)
    for c in range(NCORES):
        r = res.results[c]
        resid[SC * c:SC * (c + 1)] = r["res_chunk"]
        for hc in range(8):
            out[SC * c:SC * (c + 1), 512 * hc:512 * (hc + 1)] = r["out_chunks"][hc]
    return out.reshape(1, S, H), resid.reshape(1, S, H)
